# revision 15
# baseline (speedup 1.0000x reference)
"""Fused causal+padded attention (with attention-weight output) on 8 TRN2 cores.

Data-parallel over batch: core i handles batch element i.
Per core: q/k/v [2048,128] f32, mask [2048] i32 -> attn_vec [2048,128],
attn_weights [2048,2048].

Math per core (matches the jax reference without an explicit row-max:
scores are O(+-8) so exp() cannot overflow, and masked lanes sit at
-1e9/sqrt(D) which underflows exp() to exactly 0.0 like the reference):

  S = Q K^T ; S += -1e9*(causal | pad) ; P = exp(S/sqrt(D)) / rowsum ; O = P V

Engine plan per step t (q-block/k-block of 128):
  [k,q] orientation (k on partitions): St = K_t Q^T chunks -> ACT exp with
      per-partition pad bias -> expSt (SBUF bf16, unnormalized) -> PV matmuls
      accumulate U^T[d, q] in PSUM.
  [q,k] orientation (q on partitions): S = Q_t K^T chunks + K=1 matmul
      broadcasting the pad bias row + additive causal tile on the diagonal
      128x128 -> ACT exp with accum_out giving row sums -> reciprocal ->
      tensor_scalar normalize -> DMA the full 1MB row block of attn_weights.
      Upper-triangle zeros come from persistent pre-zeroed U tiles.
  O output: per 512-wide q chunk, copy U^T to SBUF, PE-transpose, scale by
      1/rowsum, DMA out.

All matmul operands are bf16 (PE streams 1 cyc/row and LDWEIGHTS halves vs
fp32 paths); PSUM accumulation and all outputs stay f32.
"""

import sys

for _p in ("/opt/trn_rl_repo", "/opt/pypackages"):
    if _p not in sys.path:
        sys.path.append(_p)

import numpy as np

import concourse.bass as bass
import concourse.tile as tile
from concourse import bacc, mybir
from concourse import bass_utils

S = 2048
D = 128
B = 8
T = S // D  # 16 q/k blocks of 128
NEG = -1.0e9
SCALE = 1.0 / float(np.sqrt(D))

F32 = mybir.dt.float32
BF16 = mybir.dt.bfloat16
I32 = mybir.dt.int32
AF = mybir.ActivationFunctionType
ALU = mybir.AluOpType


def _pieces(c0, c1, step=512):
    """Yield (start, width) covering [c0, c1) in <=step chunks."""
    while c0 < c1:
        w = min(step, c1 - c0)
        yield c0, w
        c0 += w


def build_nc():
    nc = bacc.Bacc("TRN2", target_bir_lowering=False, debug=False, num_devices=B)

    q_d = nc.declare_dram_parameter("query", [S, D], F32, isOutput=False).ap()
    k_d = nc.declare_dram_parameter("key", [S, D], F32, isOutput=False).ap()
    v_d = nc.declare_dram_parameter("value", [S, D], F32, isOutput=False).ap()
    m_d = nc.declare_dram_parameter("mask", [S], I32, isOutput=False).ap()
    ov_d = nc.declare_dram_parameter("out_v", [S, D], F32, isOutput=True).ap()
    ow_d = nc.declare_dram_parameter("out_w", [S, S], F32, isOutput=True).ap()

    from contextlib import ExitStack

    with tile.TileContext(nc, num_cores=B) as tc, ExitStack() as stack:
        consts = stack.enter_context(tc.tile_pool(name="consts", bufs=1))
        big = stack.enter_context(tc.tile_pool(name="big", bufs=1))

        # ---- constant tiles ----
        identity = consts.tile([128, 128], BF16, tag="identity")
        nc.vector.memset(identity, 1.0)
        # keep 1 where (p - l) == 0 else 0
        nc.gpsimd.affine_select(
            out=identity, in_=identity, pattern=[[-1, 128]], base=0,
            channel_multiplier=1, compare_op=ALU.is_equal, fill=0.0,
        )
        # additive causal tile, [q,k] diag: 0 where l <= p else -1e9
        ct_qk = consts.tile([128, 128], F32, tag="ct_qk")
        nc.vector.memset(ct_qk, 0.0)
        nc.gpsimd.affine_select(
            out=ct_qk, in_=ct_qk, pattern=[[-1, 128]], base=0,
            channel_multiplier=1, compare_op=ALU.is_ge, fill=NEG,
        )
        # multiplicative causal tile, [k,q] diag: 1 where p <= l else 0
        ct_kq = consts.tile([128, 128], BF16, tag="ct_kq")
        nc.vector.memset(ct_kq, 1.0)
        nc.gpsimd.affine_select(
            out=ct_kq, in_=ct_kq, pattern=[[1, 128]], base=0,
            channel_multiplier=-1, compare_op=ALU.is_ge, fill=0.0,
        )
        ones1 = consts.tile([1, 128], BF16, tag="ones1")
        nc.vector.memset(ones1, 1.0)

        padbias_row = consts.tile([1, S], BF16, tag="padbias_row")
        padbias_col = consts.tile([128, T], F32, tag="padbias_col")
        rs = consts.tile([128, T], F32, tag="rs")       # 1/rowsum per q block
        ssum = consts.tile([128, T], F32, tag="ssum")   # rowsum per q block
        accs = consts.tile([128, 4 * T], F32, tag="accs")  # accum_out pieces

        # ---- persistent big tiles ----
        qt = big.tile([128, S], BF16, tag="qt")  # Q^T: [d, q]
        kt = big.tile([128, S], BF16, tag="kt")  # K^T: [d, k]
        vn = big.tile([128, S], BF16, tag="vn")  # vn[p, 128b+l] = V[128b+p, l]

        u_tiles = [big.tile([128, S], F32, tag=f"u{i}", name=f"u{i}") for i in range(3)]
        for ut in u_tiles:
            nc.gpsimd.memset(ut, 0.0)

        # ---- prep: loads, mask conversion, Q/K transposes ----
        with (
            tc.tile_pool(name="prep_sb", bufs=1) as prep_sb,
            tc.tile_pool(name="prep_ps", bufs=2, space="PSUM") as prep_ps,
        ):
            # kick off input DMAs chunked per 512-column group so the
            # cast->transpose->copy pipeline starts after the first 256KB
            qstage = prep_sb.tile([128, S], F32, tag="qstage")
            kstage = prep_sb.tile([128, S], F32, tag="kstage")
            vstage = prep_sb.tile([128, S], F32, tag="vstage")
            for src, dst in ((q_d, qstage), (k_d, kstage), (v_d, vstage)):
                for g in range(4):
                    nc.sync.dma_start(
                        out=dst[:, 512 * g:512 * (g + 1)].rearrange(
                            "p (b l) -> p b l", b=4
                        ),
                        in_=src[512 * g:512 * (g + 1), :].rearrange(
                            "(b p) l -> p b l", p=128
                        ),
                    )
            m16i = prep_sb.tile([16, 128], I32, tag="m16i")
            nc.sync.dma_start(out=m16i, in_=m_d.rearrange("(p f) -> p f", p=16))

            # mask -> -1e9 * mask, in [128,16] (per-partition bias) and [1,S]
            m16f = prep_sb.tile([16, 128], F32, tag="m16f")
            nc.vector.tensor_copy(m16f, m16i)
            nc.vector.tensor_scalar_mul(m16f, m16f, NEG)
            m16b = prep_sb.tile([16, 128], BF16, tag="m16b")
            nc.vector.tensor_copy(m16b, m16f)
            mps = prep_ps.tile([128, 16], BF16, tag="mps", bufs=1)
            nc.tensor.transpose(mps, m16b, identity[0:16, 0:16])
            nc.vector.tensor_copy(padbias_col, mps)

            # flatten [16,128] -> [1,2048] across partitions via SBUF DMA
            # (a [1,2048] DVE chain would run on one lane and gate the PE
            # pipeline start)
            nc.sync.dma_start(out=padbias_row, in_=m16b)

            # Q/K: cast to bf16 per group, PE-transpose 4 blocks per psum
            # tile; casts/copies split across ACT (q) and DVE (k/v)
            qstage_b = prep_sb.tile([128, S], BF16, tag="qstage_b")
            kstage_b = prep_sb.tile([128, S], BF16, tag="kstage_b")
            for g in range(4):
                sl = slice(512 * g, 512 * (g + 1))
                nc.vector.tensor_copy(vn[:, sl], vstage[:, sl])
                nc.scalar.copy(qstage_b[:, sl], qstage[:, sl])
                nc.vector.tensor_copy(kstage_b[:, sl], kstage[:, sl])
                psq = prep_ps.tile([128, 512], BF16, tag="psq")
                psk = prep_ps.tile([128, 512], BF16, tag="psk")
                for j in range(4):
                    b = 4 * g + j
                    nc.tensor.transpose(
                        psq[:, 128 * j:128 * (j + 1)],
                        qstage_b[:, 128 * b:128 * (b + 1)],
                        identity,
                    )
                    nc.tensor.transpose(
                        psk[:, 128 * j:128 * (j + 1)],
                        kstage_b[:, 128 * b:128 * (b + 1)],
                        identity,
                    )
                nc.scalar.copy(qt[:, sl], psq)
                nc.vector.tensor_copy(kt[:, sl], psk)

        # ---- main pools ----
        with (
            tc.tile_pool(name="stp", bufs=2, space="PSUM") as stp_pool,
            tc.tile_pool(name="spp", bufs=2, space="PSUM") as sp_pool,
            tc.tile_pool(name="utp", bufs=4, space="PSUM") as ut_pool,
            tc.tile_pool(name="expst", bufs=4) as expst_pool,
            tc.tile_pool(name="osb", bufs=2) as osb_pool,
        ):
            ut_psum = [None] * 4

            for t in range(T):
                qc0 = t // 4  # first valid 512-wide q chunk on the [k,q] side
                start_a = 128 * t

                # ======== phase A: [k,q] St row for k-block t ========
                exp_t = expst_pool.tile([128, S], BF16, tag="expst", name=f"expst{t}")
                if start_a > 512 * qc0:
                    # PV reads chunk qc0 from 512*qc0; zero the causally-dead
                    # prefix the ACT below does not write.
                    nc.vector.memset(exp_t[:, 512 * qc0:start_a], 0.0)
                for c0, w in _pieces(start_a, S):
                    stp = stp_pool.tile([128, 512], F32, tag="stp")
                    nc.tensor.matmul(
                        stp[:, :w],
                        lhsT=kt[:, start_a:start_a + 128],
                        rhs=qt[:, c0:c0 + w],
                        start=True, stop=True,
                    )
                    nc.scalar.activation(
                        out=exp_t[:, c0:c0 + w], in_=stp[:, :w], func=AF.Exp,
                        bias=padbias_col[:, t:t + 1], scale=SCALE,
                    )
                # causal zeros on the leading diagonal 128x128
                nc.gpsimd.tensor_mul(
                    exp_t[:, start_a:start_a + 128],
                    exp_t[:, start_a:start_a + 128],
                    ct_kq,
                )

                # ======== PV accumulation: U^T[d, q] += V_t^T expSt ========
                for qc in range(qc0, 4):
                    if t == 0:
                        ut_psum[qc] = ut_pool.tile(
                            [128, 512], F32, tag="ut", name=f"ut{qc}"
                        )
                    nc.tensor.matmul(
                        ut_psum[qc],
                        lhsT=vn[:, start_a:start_a + 128],
                        rhs=exp_t[:, 512 * qc:512 * (qc + 1)],
                        start=(t == 0), stop=(t == 4 * qc + 3),
                        skip_group_check=True,
                    )

                # ======== phase B: [q,k] S row for q-block t ========
                wb = 128 * (t + 1)
                u_t = u_tiles[t % 3]
                npieces = 0
                for c0, w in _pieces(0, wb):
                    i = npieces
                    npieces += 1
                    sp = sp_pool.tile([128, 512], F32, tag="sp")
                    nc.tensor.matmul(
                        sp[:, :w],
                        lhsT=qt[:, start_a:start_a + 128],
                        rhs=kt[:, c0:c0 + w],
                        start=True, stop=False,
                        skip_group_check=True,
                    )
                    nc.tensor.matmul(
                        sp[:, :w],
                        lhsT=ones1,
                        rhs=padbias_row[:, c0:c0 + w],
                        start=False, stop=True,
                        skip_group_check=True,
                    )
                    if c0 <= start_a < c0 + w:
                        off = start_a - c0
                        nc.vector.tensor_add(
                            sp[:, off:off + 128], sp[:, off:off + 128], ct_qk
                        )
                    nc.scalar.activation(
                        out=u_t[:, c0:c0 + w], in_=sp[:, :w], func=AF.Exp,
                        bias=0.0, scale=SCALE,
                        accum_out=accs[:, 4 * t + i:4 * t + i + 1],
                    )
                nc.vector.reduce_sum(
                    ssum[:, t:t + 1], accs[:, 4 * t:4 * t + npieces],
                    axis=mybir.AxisListType.X,
                )
                nc.vector.reciprocal(rs[:, t:t + 1], ssum[:, t:t + 1])
                nc.gpsimd.tensor_scalar_mul(u_t[:, 0:wb], u_t[:, 0:wb], rs[:, t:t + 1])
                nc.sync.dma_start(
                    out=ow_d[start_a:start_a + 128, :], in_=u_t[:, :]
                )

                # ======== O output for finished q chunks ========
                if t % 4 == 3:
                    qc = t // 4
                    uts = osb_pool.tile([128, 512], BF16, tag="uts")
                    nc.vector.tensor_copy(uts, ut_psum[qc])
                    otr = ut_pool.tile([128, 512], BF16, tag="ut", name=f"otr{qc}")
                    for j in range(4):
                        nc.tensor.transpose(
                            otr[:, 128 * j:128 * (j + 1)],
                            uts[:, 128 * j:128 * (j + 1)],
                            identity,
                        )
                    osb = osb_pool.tile([128, 512], F32, tag="osb")
                    for j in range(4):
                        tb = 4 * qc + j
                        nc.vector.tensor_scalar_mul(
                            osb[:, 128 * j:128 * (j + 1)],
                            otr[:, 128 * j:128 * (j + 1)],
                            rs[:, tb:tb + 1],
                        )
                    nc.sync.dma_start(
                        out=ov_d[512 * qc:512 * (qc + 1), :].rearrange(
                            "(j p) l -> p j l", p=128
                        ),
                        in_=osb.rearrange("p (j l) -> p j l", j=4),
                    )

    nc.compile()
    return nc


_NC = None


def _get_nc():
    global _NC
    if _NC is None:
        _NC = build_nc()
    return _NC


def run(query, key, value, mask, trace=False):
    nc = _get_nc()
    in_maps = [
        {
            "query": np.ascontiguousarray(query[i], dtype=np.float32),
            "key": np.ascontiguousarray(key[i], dtype=np.float32),
            "value": np.ascontiguousarray(value[i], dtype=np.float32),
            "mask": np.ascontiguousarray(mask[i], dtype=np.int32),
        }
        for i in range(B)
    ]
    res = bass_utils.run_bass_kernel_spmd(
        nc, in_maps, core_ids=list(range(B)), trace=trace
    )
    attn_vec = np.stack([res.results[i]["out_v"] for i in range(B)])
    attn_w = np.stack([res.results[i]["out_w"] for i in range(B)])
    return (attn_vec, attn_w), res


def kernel(query, key, value, mask):
    (attn_vec, attn_w), _ = run(query, key, value, mask)
    return attn_vec.astype(np.float32), attn_w.astype(np.float32)


# revision 16
# speedup vs baseline: 2.9919x; 2.9919x over previous
"""Fused causal+padded attention (with attention-weight output) on 8 TRN2 cores.

Data-parallel over batch: core i handles batch element i.
Per core: q/k/v [2048,128] f32, mask [2048] i32 -> attn_vec [2048,128],
attn_weights [2048,2048].

Math per core (matches the jax reference without an explicit row-max:
scores are O(+-8) so exp() cannot overflow, and masked lanes sit at
-1e9/sqrt(D) which underflows exp() to exactly 0.0 like the reference):

  S = Q K^T ; S += -1e9*(causal | pad) ; P = exp(S/sqrt(D)) / rowsum ; O = P V

Engine plan per step t (q-block/k-block of 128):
  [k,q] orientation (k on partitions): St = K_t Q^T chunks -> ACT exp with
      per-partition pad bias -> expSt (SBUF bf16, unnormalized) -> PV matmuls
      accumulate U^T[d, q] in PSUM.
  [q,k] orientation (q on partitions): S = Q_t K^T chunks + K=1 matmul
      broadcasting the pad bias row + additive causal tile on the diagonal
      128x128 -> ACT exp with accum_out giving row sums -> reciprocal ->
      tensor_scalar normalize -> DMA the full 1MB row block of attn_weights.
      Upper-triangle zeros come from persistent pre-zeroed U tiles.
  O output: per 512-wide q chunk, copy U^T to SBUF, PE-transpose, scale by
      1/rowsum, DMA out.

All matmul operands are bf16 (PE streams 1 cyc/row and LDWEIGHTS halves vs
fp32 paths); PSUM accumulation and all outputs stay f32.
"""

import sys

for _p in ("/opt/trn_rl_repo", "/opt/pypackages"):
    if _p not in sys.path:
        sys.path.append(_p)

import numpy as np

import concourse.bass as bass
import concourse.tile as tile
from concourse import bacc, mybir
from concourse import bass_utils

S = 2048
D = 128
B = 8
T = S // D  # 16 q/k blocks of 128
NEG = -1.0e9
SCALE = 1.0 / float(np.sqrt(D))

F32 = mybir.dt.float32
BF16 = mybir.dt.bfloat16
I32 = mybir.dt.int32
AF = mybir.ActivationFunctionType
ALU = mybir.AluOpType


def _pieces(c0, c1, step=512):
    """Yield (start, width) covering [c0, c1) in <=step chunks."""
    while c0 < c1:
        w = min(step, c1 - c0)
        yield c0, w
        c0 += w


def build_nc():
    nc = bacc.Bacc("TRN2", target_bir_lowering=False, debug=False, num_devices=B)

    q_d = nc.declare_dram_parameter("query", [S, D], F32, isOutput=False).ap()
    k_d = nc.declare_dram_parameter("key", [S, D], F32, isOutput=False).ap()
    v_d = nc.declare_dram_parameter("value", [S, D], F32, isOutput=False).ap()
    m_d = nc.declare_dram_parameter("mask", [S], I32, isOutput=False).ap()
    ov_d = nc.declare_dram_parameter("out_v", [S, D], F32, isOutput=True).ap()
    ow_d = nc.declare_dram_parameter("out_w", [S, S], F32, isOutput=True).ap()

    from contextlib import ExitStack

    with tile.TileContext(nc, num_cores=B) as tc, ExitStack() as stack:
        consts = stack.enter_context(tc.tile_pool(name="consts", bufs=1))
        big = stack.enter_context(tc.tile_pool(name="big", bufs=1))

        # ---- constant tiles ----
        identity = consts.tile([128, 128], BF16, tag="identity")
        nc.vector.memset(identity, 1.0)
        # keep 1 where (p - l) == 0 else 0
        nc.gpsimd.affine_select(
            out=identity, in_=identity, pattern=[[-1, 128]], base=0,
            channel_multiplier=1, compare_op=ALU.is_equal, fill=0.0,
        )
        # additive causal tile, [q,k] diag: 0 where l <= p else -1e9
        ct_qk = consts.tile([128, 128], F32, tag="ct_qk")
        nc.vector.memset(ct_qk, 0.0)
        nc.gpsimd.affine_select(
            out=ct_qk, in_=ct_qk, pattern=[[-1, 128]], base=0,
            channel_multiplier=1, compare_op=ALU.is_ge, fill=NEG,
        )
        # multiplicative causal tile, [k,q] diag: 1 where p <= l else 0
        ct_kq = consts.tile([128, 128], BF16, tag="ct_kq")
        nc.vector.memset(ct_kq, 1.0)
        nc.gpsimd.affine_select(
            out=ct_kq, in_=ct_kq, pattern=[[1, 128]], base=0,
            channel_multiplier=-1, compare_op=ALU.is_ge, fill=0.0,
        )
        ones1 = consts.tile([1, 128], BF16, tag="ones1")
        nc.vector.memset(ones1, 1.0)

        padbias_row = consts.tile([1, S], BF16, tag="padbias_row")
        padbias_col = consts.tile([128, T], F32, tag="padbias_col")
        rs = consts.tile([128, T], F32, tag="rs")       # 1/rowsum per q block
        ssum = consts.tile([128, T], F32, tag="ssum")   # rowsum per q block
        accs = consts.tile([128, 4 * T], F32, tag="accs")  # accum_out pieces

        # ---- persistent big tiles ----
        qt = big.tile([128, S], BF16, tag="qt")  # Q^T: [d, q]
        kt = big.tile([128, S], BF16, tag="kt")  # K^T: [d, k]
        vn = big.tile([128, S], BF16, tag="vn")  # vn[p, 128b+l] = V[128b+p, l]

        u_tiles = [big.tile([128, S], F32, tag=f"u{i}", name=f"u{i}") for i in range(3)]
        for ut in u_tiles:
            nc.gpsimd.memset(ut, 0.0)

        # ---- prep: loads, mask conversion, Q/K transposes ----
        with (
            tc.tile_pool(name="prep_sb", bufs=1) as prep_sb,
            tc.tile_pool(name="prep_ps", bufs=2, space="PSUM") as prep_ps,
        ):
            # kick off input DMAs chunked per 512-column group so the
            # cast->transpose->copy pipeline starts after the first 256KB
            qstage = prep_sb.tile([128, S], F32, tag="qstage")
            kstage = prep_sb.tile([128, S], F32, tag="kstage")
            vstage = prep_sb.tile([128, S], F32, tag="vstage")
            for src, dst in ((q_d, qstage), (k_d, kstage), (v_d, vstage)):
                for g in range(4):
                    nc.sync.dma_start(
                        out=dst[:, 512 * g:512 * (g + 1)].rearrange(
                            "p (b l) -> p b l", b=4
                        ),
                        in_=src[512 * g:512 * (g + 1), :].rearrange(
                            "(b p) l -> p b l", p=128
                        ),
                    )
            m16i = prep_sb.tile([16, 128], I32, tag="m16i")
            nc.sync.dma_start(out=m16i, in_=m_d.rearrange("(p f) -> p f", p=16))

            # mask -> -1e9 * mask, in [128,16] (per-partition bias) and [1,S]
            m16f = prep_sb.tile([16, 128], F32, tag="m16f")
            nc.vector.tensor_copy(m16f, m16i)
            nc.vector.tensor_scalar_mul(m16f, m16f, NEG)
            m16b = prep_sb.tile([16, 128], BF16, tag="m16b")
            nc.vector.tensor_copy(m16b, m16f)
            mps = prep_ps.tile([128, 16], BF16, tag="mps", bufs=1)
            nc.tensor.transpose(mps, m16b, identity[0:16, 0:16])
            nc.vector.tensor_copy(padbias_col, mps)

            # flatten [16,128] -> [1,2048] across partitions via SBUF DMA
            # (a [1,2048] DVE chain would run on one lane and gate the PE
            # pipeline start)
            nc.sync.dma_start(out=padbias_row, in_=m16b)

            # Q/K: cast to bf16 per group, PE-transpose 4 blocks per psum
            # tile; casts/copies split across ACT (q) and DVE (k/v)
            qstage_b = prep_sb.tile([128, S], BF16, tag="qstage_b")
            kstage_b = prep_sb.tile([128, S], BF16, tag="kstage_b")
            for g in range(4):
                sl = slice(512 * g, 512 * (g + 1))
                nc.vector.tensor_copy(vn[:, sl], vstage[:, sl])
                nc.scalar.copy(qstage_b[:, sl], qstage[:, sl])
                nc.vector.tensor_copy(kstage_b[:, sl], kstage[:, sl])
                psq = prep_ps.tile([128, 512], BF16, tag="psq")
                psk = prep_ps.tile([128, 512], BF16, tag="psk")
                for j in range(4):
                    b = 4 * g + j
                    nc.tensor.transpose(
                        psq[:, 128 * j:128 * (j + 1)],
                        qstage_b[:, 128 * b:128 * (b + 1)],
                        identity,
                    )
                    nc.tensor.transpose(
                        psk[:, 128 * j:128 * (j + 1)],
                        kstage_b[:, 128 * b:128 * (b + 1)],
                        identity,
                    )
                nc.scalar.copy(qt[:, sl], psq)
                nc.vector.tensor_copy(kt[:, sl], psk)

        # ---- main pools ----
        with (
            tc.tile_pool(name="stp", bufs=2, space="PSUM") as stp_pool,
            tc.tile_pool(name="spp", bufs=2, space="PSUM") as sp_pool,
            tc.tile_pool(name="utp", bufs=4, space="PSUM") as ut_pool,
            tc.tile_pool(name="expst", bufs=4) as expst_pool,
            tc.tile_pool(name="osb", bufs=2) as osb_pool,
        ):
            ut_psum = [None] * 4

            for t in range(T):
                qc0 = t // 4  # first valid 512-wide q chunk on the [k,q] side
                start_a = 128 * t

                # ======== phase A: [k,q] St row for k-block t ========
                exp_t = expst_pool.tile([128, S], BF16, tag="expst", name=f"expst{t}")
                if start_a > 512 * qc0:
                    # PV reads chunk qc0 from 512*qc0; zero the causally-dead
                    # prefix the ACT below does not write.
                    nc.vector.memset(exp_t[:, 512 * qc0:start_a], 0.0)
                for c0, w in _pieces(start_a, S):
                    stp = stp_pool.tile([128, 512], F32, tag="stp")
                    nc.tensor.matmul(
                        stp[:, :w],
                        lhsT=kt[:, start_a:start_a + 128],
                        rhs=qt[:, c0:c0 + w],
                        start=True, stop=True,
                    )
                    nc.scalar.activation(
                        out=exp_t[:, c0:c0 + w], in_=stp[:, :w], func=AF.Exp,
                        bias=padbias_col[:, t:t + 1], scale=SCALE,
                    )
                # causal zeros on the leading diagonal 128x128
                nc.gpsimd.tensor_mul(
                    exp_t[:, start_a:start_a + 128],
                    exp_t[:, start_a:start_a + 128],
                    ct_kq,
                )

                # ======== PV accumulation: U^T[d, q] += V_t^T expSt ========
                for qc in range(qc0, 4):
                    if t == 0:
                        ut_psum[qc] = ut_pool.tile(
                            [128, 512], F32, tag="ut", name=f"ut{qc}"
                        )
                    nc.tensor.matmul(
                        ut_psum[qc],
                        lhsT=vn[:, start_a:start_a + 128],
                        rhs=exp_t[:, 512 * qc:512 * (qc + 1)],
                        start=(t == 0), stop=(t == 4 * qc + 3),
                        skip_group_check=True,
                    )

                # ======== phase B: [q,k] S row for q-block t ========
                wb = 128 * (t + 1)
                u_t = u_tiles[t % 3]
                npieces = 0
                for p0, pw in _pieces(0, wb, 1024):
                    i = npieces
                    npieces += 1
                    sp = sp_pool.tile([128, 1024], F32, tag="sp", bufs=1)
                    for c0, w in _pieces(p0, p0 + pw):
                        o = c0 - p0
                        nc.tensor.matmul(
                            sp[:, o:o + w],
                            lhsT=qt[:, start_a:start_a + 128],
                            rhs=kt[:, c0:c0 + w],
                            start=True, stop=False,
                            skip_group_check=True,
                        )
                        nc.tensor.matmul(
                            sp[:, o:o + w],
                            lhsT=ones1,
                            rhs=padbias_row[:, c0:c0 + w],
                            start=False, stop=True,
                            skip_group_check=True,
                        )
                    if p0 <= start_a < p0 + pw:
                        off = start_a - p0
                        nc.vector.tensor_add(
                            sp[:, off:off + 128], sp[:, off:off + 128], ct_qk
                        )
                    nc.scalar.activation(
                        out=u_t[:, p0:p0 + pw], in_=sp[:, :pw], func=AF.Exp,
                        bias=0.0, scale=SCALE,
                        accum_out=accs[:, 4 * t + i:4 * t + i + 1],
                    )
                nc.vector.reduce_sum(
                    ssum[:, t:t + 1], accs[:, 4 * t:4 * t + npieces],
                    axis=mybir.AxisListType.X,
                )
                nc.vector.reciprocal(rs[:, t:t + 1], ssum[:, t:t + 1])
                nc.vector.tensor_scalar_mul(u_t[:, 0:wb], u_t[:, 0:wb], rs[:, t:t + 1])
                nc.sync.dma_start(
                    out=ow_d[start_a:start_a + 128, :], in_=u_t[:, :]
                )

                # ======== O output for finished q chunks ========
                if t % 4 == 3:
                    qc = t // 4
                    uts = osb_pool.tile([128, 512], BF16, tag="uts")
                    nc.vector.tensor_copy(uts, ut_psum[qc])
                    otr = ut_pool.tile([128, 512], BF16, tag="ut", name=f"otr{qc}")
                    for j in range(4):
                        nc.tensor.transpose(
                            otr[:, 128 * j:128 * (j + 1)],
                            uts[:, 128 * j:128 * (j + 1)],
                            identity,
                        )
                    osb = osb_pool.tile([128, 512], F32, tag="osb")
                    for j in range(4):
                        tb = 4 * qc + j
                        nc.vector.tensor_scalar_mul(
                            osb[:, 128 * j:128 * (j + 1)],
                            otr[:, 128 * j:128 * (j + 1)],
                            rs[:, tb:tb + 1],
                        )
                    nc.sync.dma_start(
                        out=ov_d[512 * qc:512 * (qc + 1), :].rearrange(
                            "(j p) l -> p j l", p=128
                        ),
                        in_=osb.rearrange("p (j l) -> p j l", j=4),
                    )

    nc.compile()
    return nc


_NC = None


def _get_nc():
    global _NC
    if _NC is None:
        _NC = build_nc()
    return _NC


def run(query, key, value, mask, trace=False):
    nc = _get_nc()
    in_maps = [
        {
            "query": np.ascontiguousarray(query[i], dtype=np.float32),
            "key": np.ascontiguousarray(key[i], dtype=np.float32),
            "value": np.ascontiguousarray(value[i], dtype=np.float32),
            "mask": np.ascontiguousarray(mask[i], dtype=np.int32),
        }
        for i in range(B)
    ]
    res = bass_utils.run_bass_kernel_spmd(
        nc, in_maps, core_ids=list(range(B)), trace=trace
    )
    attn_vec = np.stack([res.results[i]["out_v"] for i in range(B)])
    attn_w = np.stack([res.results[i]["out_w"] for i in range(B)])
    return (attn_vec, attn_w), res


def kernel(query, key, value, mask):
    (attn_vec, attn_w), _ = run(query, key, value, mask)
    return attn_vec.astype(np.float32), attn_w.astype(np.float32)


# revision 17
# speedup vs baseline: 4.0113x; 1.3407x over previous
"""Fused causal+padded attention (with attention-weight output) on 8 TRN2 cores.

Data-parallel over batch: core i handles batch element i.
Per core: q/k/v [2048,128] f32, mask [2048] i32 -> attn_vec [2048,128],
attn_weights [2048,2048].

Math per core (matches the jax reference without an explicit row-max:
scores are O(+-8) so exp() cannot overflow, and masked lanes sit at
-1e9/sqrt(D) which underflows exp() to exactly 0.0 like the reference):

  S = Q K^T ; S += -1e9*(causal | pad) ; P = exp(S/sqrt(D)) / rowsum ; O = P V

Engine plan per step t (q-block/k-block of 128):
  [k,q] orientation (k on partitions): St = K_t Q^T chunks -> ACT exp with
      per-partition pad bias -> expSt (SBUF bf16, unnormalized) -> PV matmuls
      accumulate U^T[d, q] in PSUM.
  [q,k] orientation (q on partitions): S = Q_t K^T chunks + K=1 matmul
      broadcasting the pad bias row + additive causal tile on the diagonal
      128x128 -> ACT exp with accum_out giving row sums -> reciprocal ->
      tensor_scalar normalize -> DMA the full 1MB row block of attn_weights.
      Upper-triangle zeros come from persistent pre-zeroed U tiles.
  O output: per 512-wide q chunk, copy U^T to SBUF, PE-transpose, scale by
      1/rowsum, DMA out.

All matmul operands are bf16 (PE streams 1 cyc/row and LDWEIGHTS halves vs
fp32 paths); PSUM accumulation and all outputs stay f32.
"""

import sys

for _p in ("/opt/trn_rl_repo", "/opt/pypackages"):
    if _p not in sys.path:
        sys.path.append(_p)

import numpy as np

import concourse.bass as bass
import concourse.tile as tile
from concourse import bacc, mybir
from concourse import bass_utils

S = 2048
D = 128
B = 8
T = S // D  # 16 q/k blocks of 128
NEG = -1.0e9
SCALE = 1.0 / float(np.sqrt(D))

F32 = mybir.dt.float32
BF16 = mybir.dt.bfloat16
I32 = mybir.dt.int32
AF = mybir.ActivationFunctionType
ALU = mybir.AluOpType


def _pieces(c0, c1, step=512):
    """Yield (start, width) covering [c0, c1) in <=step chunks."""
    while c0 < c1:
        w = min(step, c1 - c0)
        yield c0, w
        c0 += w


def build_nc():
    nc = bacc.Bacc("TRN2", target_bir_lowering=False, debug=False, num_devices=B)

    q_d = nc.declare_dram_parameter("query", [S, D], F32, isOutput=False).ap()
    k_d = nc.declare_dram_parameter("key", [S, D], F32, isOutput=False).ap()
    v_d = nc.declare_dram_parameter("value", [S, D], F32, isOutput=False).ap()
    m_d = nc.declare_dram_parameter("mask", [S], I32, isOutput=False).ap()
    ov_d = nc.declare_dram_parameter("out_v", [S, D], F32, isOutput=True).ap()
    ow_d = nc.declare_dram_parameter("out_w", [S, S], F32, isOutput=True).ap()

    from contextlib import ExitStack

    with tile.TileContext(nc, num_cores=B) as tc, ExitStack() as stack:
        consts = stack.enter_context(tc.tile_pool(name="consts", bufs=1))
        big = stack.enter_context(tc.tile_pool(name="big", bufs=1))

        # ---- constant tiles ----
        identity = consts.tile([128, 128], BF16, tag="identity")
        nc.vector.memset(identity, 1.0)
        # keep 1 where (p - l) == 0 else 0
        nc.gpsimd.affine_select(
            out=identity, in_=identity, pattern=[[-1, 128]], base=0,
            channel_multiplier=1, compare_op=ALU.is_equal, fill=0.0,
        )
        # additive causal tile, [q,k] diag: 0 where l <= p else -1e9
        ct_qk = consts.tile([128, 128], F32, tag="ct_qk")
        nc.vector.memset(ct_qk, 0.0)
        nc.gpsimd.affine_select(
            out=ct_qk, in_=ct_qk, pattern=[[-1, 128]], base=0,
            channel_multiplier=1, compare_op=ALU.is_ge, fill=NEG,
        )
        # multiplicative causal tile, [k,q] diag: 1 where p <= l else 0
        ct_kq = consts.tile([128, 128], BF16, tag="ct_kq")
        nc.vector.memset(ct_kq, 1.0)
        nc.gpsimd.affine_select(
            out=ct_kq, in_=ct_kq, pattern=[[1, 128]], base=0,
            channel_multiplier=-1, compare_op=ALU.is_ge, fill=0.0,
        )
        ones1 = consts.tile([1, 128], BF16, tag="ones1")
        nc.vector.memset(ones1, 1.0)

        padbias_row = consts.tile([1, S], BF16, tag="padbias_row")
        padbias_col = consts.tile([128, T], F32, tag="padbias_col")
        rs = consts.tile([128, T], F32, tag="rs")       # 1/rowsum per q block
        ssum = consts.tile([128, T], F32, tag="ssum")   # rowsum per q block
        accs = consts.tile([128, 4 * T], F32, tag="accs")  # accum_out pieces

        # ---- persistent big tiles ----
        qt = big.tile([128, S], BF16, tag="qt")  # Q^T: [d, q]
        kt = big.tile([128, S], BF16, tag="kt")  # K^T: [d, k]
        vn = big.tile([128, S], BF16, tag="vn")  # vn[p, 128b+l] = V[128b+p, l]

        u_tiles = [big.tile([128, S], F32, tag=f"u{i}", name=f"u{i}") for i in range(3)]
        for ut in u_tiles:
            nc.gpsimd.memset(ut, 0.0)

        # ---- prep: loads, mask conversion, Q/K transposes ----
        with (
            tc.tile_pool(name="prep_sb", bufs=1) as prep_sb,
            tc.tile_pool(name="prep_ps", bufs=2, space="PSUM") as prep_ps,
        ):
            # kick off input DMAs chunked per 512-column group so the
            # cast->transpose->copy pipeline starts after the first 256KB
            qstage = prep_sb.tile([128, S], F32, tag="qstage")
            kstage = prep_sb.tile([128, S], F32, tag="kstage")
            vstage = prep_sb.tile([128, S], F32, tag="vstage")
            for src, dst in ((q_d, qstage), (k_d, kstage), (v_d, vstage)):
                for g in range(4):
                    nc.sync.dma_start(
                        out=dst[:, 512 * g:512 * (g + 1)].rearrange(
                            "p (b l) -> p b l", b=4
                        ),
                        in_=src[512 * g:512 * (g + 1), :].rearrange(
                            "(b p) l -> p b l", p=128
                        ),
                    )
            m16i = prep_sb.tile([16, 128], I32, tag="m16i")
            nc.sync.dma_start(out=m16i, in_=m_d.rearrange("(p f) -> p f", p=16))

            # mask -> -1e9 * mask, in [128,16] (per-partition bias) and [1,S]
            m16f = prep_sb.tile([16, 128], F32, tag="m16f")
            nc.vector.tensor_copy(m16f, m16i)
            nc.vector.tensor_scalar_mul(m16f, m16f, NEG)
            m16b = prep_sb.tile([16, 128], BF16, tag="m16b")
            nc.vector.tensor_copy(m16b, m16f)
            mps = prep_ps.tile([128, 16], BF16, tag="mps", bufs=1)
            nc.tensor.transpose(mps, m16b, identity[0:16, 0:16])
            nc.vector.tensor_copy(padbias_col, mps)

            # flatten [16,128] -> [1,2048] across partitions via SBUF DMA
            # (a [1,2048] DVE chain would run on one lane and gate the PE
            # pipeline start)
            nc.sync.dma_start(out=padbias_row, in_=m16b)

            # Q/K: cast to bf16 per group, PE-transpose 4 blocks per psum
            # tile; casts/copies split across ACT (q) and DVE (k/v)
            qstage_b = prep_sb.tile([128, S], BF16, tag="qstage_b")
            kstage_b = prep_sb.tile([128, S], BF16, tag="kstage_b")
            for g in range(4):
                sl = slice(512 * g, 512 * (g + 1))
                nc.vector.tensor_copy(vn[:, sl], vstage[:, sl])
                nc.scalar.copy(qstage_b[:, sl], qstage[:, sl])
                nc.vector.tensor_copy(kstage_b[:, sl], kstage[:, sl])
                psq = prep_ps.tile([128, 512], BF16, tag="psq")
                psk = prep_ps.tile([128, 512], BF16, tag="psk")
                for j in range(4):
                    b = 4 * g + j
                    nc.tensor.transpose(
                        psq[:, 128 * j:128 * (j + 1)],
                        qstage_b[:, 128 * b:128 * (b + 1)],
                        identity,
                    )
                    nc.tensor.transpose(
                        psk[:, 128 * j:128 * (j + 1)],
                        kstage_b[:, 128 * b:128 * (b + 1)],
                        identity,
                    )
                nc.scalar.copy(qt[:, sl], psq)
                nc.vector.tensor_copy(kt[:, sl], psk)

        # ---- main pools ----
        with (
            tc.tile_pool(name="stp", bufs=2, space="PSUM") as stp_pool,
            tc.tile_pool(name="spp", bufs=2, space="PSUM") as sp_pool,
            tc.tile_pool(name="utp", bufs=4, space="PSUM") as ut_pool,
            tc.tile_pool(name="expst", bufs=4) as expst_pool,
            tc.tile_pool(name="osb", bufs=2) as osb_pool,
        ):
            ut_psum = [None] * 4

            for t in range(T):
                qc0 = t // 4  # first valid 512-wide q chunk on the [k,q] side
                start_a = 128 * t

                # ======== phase A: [k,q] St row for k-block t ========
                exp_t = expst_pool.tile([128, S], BF16, tag="expst", name=f"expst{t}")
                if start_a > 512 * qc0:
                    # PV reads chunk qc0 from 512*qc0; zero the causally-dead
                    # prefix the ACT below does not write.
                    nc.vector.memset(exp_t[:, 512 * qc0:start_a], 0.0)
                for c0, w in _pieces(start_a, S):
                    stp = stp_pool.tile([128, 512], F32, tag="stp")
                    nc.tensor.matmul(
                        stp[:, :w],
                        lhsT=kt[:, start_a:start_a + 128],
                        rhs=qt[:, c0:c0 + w],
                        start=True, stop=True,
                    )
                    nc.scalar.activation(
                        out=exp_t[:, c0:c0 + w], in_=stp[:, :w], func=AF.Exp,
                        bias=padbias_col[:, t:t + 1], scale=SCALE,
                    )
                # causal zeros on the leading diagonal 128x128
                nc.gpsimd.tensor_mul(
                    exp_t[:, start_a:start_a + 128],
                    exp_t[:, start_a:start_a + 128],
                    ct_kq,
                )

                # ======== PV accumulation: U^T[d, q] += V_t^T expSt ========
                for qc in range(qc0, 4):
                    if t == 0:
                        ut_psum[qc] = ut_pool.tile(
                            [128, 512], F32, tag="ut", name=f"ut{qc}"
                        )
                    nc.tensor.matmul(
                        ut_psum[qc],
                        lhsT=vn[:, start_a:start_a + 128],
                        rhs=exp_t[:, 512 * qc:512 * (qc + 1)],
                        start=(t == 0), stop=(t == 4 * qc + 3),
                        skip_group_check=True,
                    )

                # ======== phase B: [q,k] S row for q-block t ========
                wb = 128 * (t + 1)
                u_t = u_tiles[t % 3]
                npieces = 0
                for c0, w in _pieces(0, wb):
                    i = npieces
                    npieces += 1
                    sp = sp_pool.tile([128, 512], F32, tag="sp")
                    nc.tensor.matmul(
                        sp[:, :w],
                        lhsT=qt[:, start_a:start_a + 128],
                        rhs=kt[:, c0:c0 + w],
                        start=True, stop=False,
                        skip_group_check=True,
                    )
                    nc.tensor.matmul(
                        sp[:, :w],
                        lhsT=ones1,
                        rhs=padbias_row[:, c0:c0 + w],
                        start=False, stop=True,
                        skip_group_check=True,
                    )
                    if c0 <= start_a < c0 + w:
                        off = start_a - c0
                        nc.vector.tensor_add(
                            sp[:, off:off + 128], sp[:, off:off + 128], ct_qk
                        )
                    nc.scalar.activation(
                        out=u_t[:, c0:c0 + w], in_=sp[:, :w], func=AF.Exp,
                        bias=0.0, scale=SCALE,
                        accum_out=accs[:, 4 * t + i:4 * t + i + 1],
                    )
                nc.vector.reduce_sum(
                    ssum[:, t:t + 1], accs[:, 4 * t:4 * t + npieces],
                    axis=mybir.AxisListType.X,
                )
                nc.vector.reciprocal(rs[:, t:t + 1], ssum[:, t:t + 1])
                nc.vector.tensor_scalar_mul(u_t[:, 0:wb], u_t[:, 0:wb], rs[:, t:t + 1])
                nc.sync.dma_start(
                    out=ow_d[start_a:start_a + 128, :], in_=u_t[:, :]
                )

                # ======== O output for finished q chunks ========
                if t % 4 == 3:
                    qc = t // 4
                    uts = osb_pool.tile([128, 512], BF16, tag="uts")
                    nc.vector.tensor_copy(uts, ut_psum[qc])
                    otr = ut_pool.tile([128, 512], BF16, tag="ut", name=f"otr{qc}")
                    for j in range(4):
                        nc.tensor.transpose(
                            otr[:, 128 * j:128 * (j + 1)],
                            uts[:, 128 * j:128 * (j + 1)],
                            identity,
                        )
                    osb = osb_pool.tile([128, 512], F32, tag="osb")
                    for j in range(4):
                        tb = 4 * qc + j
                        nc.vector.tensor_scalar_mul(
                            osb[:, 128 * j:128 * (j + 1)],
                            otr[:, 128 * j:128 * (j + 1)],
                            rs[:, tb:tb + 1],
                        )
                    nc.sync.dma_start(
                        out=ov_d[512 * qc:512 * (qc + 1), :].rearrange(
                            "(j p) l -> p j l", p=128
                        ),
                        in_=osb.rearrange("p (j l) -> p j l", j=4),
                    )

    nc.compile()
    return nc


_NC = None


def _get_nc():
    global _NC
    if _NC is None:
        _NC = build_nc()
    return _NC


def run(query, key, value, mask, trace=False):
    nc = _get_nc()
    in_maps = [
        {
            "query": np.ascontiguousarray(query[i], dtype=np.float32),
            "key": np.ascontiguousarray(key[i], dtype=np.float32),
            "value": np.ascontiguousarray(value[i], dtype=np.float32),
            "mask": np.ascontiguousarray(mask[i], dtype=np.int32),
        }
        for i in range(B)
    ]
    res = bass_utils.run_bass_kernel_spmd(
        nc, in_maps, core_ids=list(range(B)), trace=trace
    )
    attn_vec = np.stack([res.results[i]["out_v"] for i in range(B)])
    attn_w = np.stack([res.results[i]["out_w"] for i in range(B)])
    return (attn_vec, attn_w), res


def kernel(query, key, value, mask):
    (attn_vec, attn_w), _ = run(query, key, value, mask)
    return attn_vec.astype(np.float32), attn_w.astype(np.float32)
